# revision 25
# baseline (speedup 1.0000x reference)
"""Distributed Trainium2 kernel for nn_Attention_73675868995842.

Sharding: 8-way head tensor-parallel (2 q heads + 1 kv head per core); hidden
states replicated (host-staged, pre-transposed + tiled); each core computes a
full [S, HID] partial of the output projection; host sums the 8 partials.

Per-core pipeline (raw Bass, hand-scheduled semaphores):
  P: qkv = hsT.T @ wqkv (fp32r)   -> rmsnorm(q,k) -> rope -> PE-transpose
     q,k to [d, s] layout; v cast to bf16.
  A (two-orientation softmax, no probs transposes):
     pass1: scores[q,k] = qT.T @ kT -> DVE row-max (negated) -> per-qtile
            negm packed + PE-transposed to a row vector [1, 512].
     pass2: scoresT[k,q] = kT.T @ qT accumulated with a rank-1 matmul
            ones[1,128] x negm_row[1,512] (bias in PSUM) -> Scalar exp ->
            probsT bf16.  GPSIMD tree-sums probsT chunks -> racc; PE
            ones-matmul collapses partitions -> r; DVE reciprocal; PE rank-1
            broadcast -> rinv[128,512]; attnT = av_psum * rinv (DVE, one
            multiply per qtile at PSUM readout).
     AV: attnT_psum += v.T @ probsT (bf16).
  O: out_partial = attnT.T @ wo (bf16) -> bf16 partial to DRAM.
"""
import contextlib
import numpy as np
import ml_dtypes

import concourse.bass as bass
from concourse import mybir
from concourse import bass_utils

F32 = mybir.dt.float32
F32R = mybir.dt.float32r
BF16 = mybir.dt.bfloat16
AX = mybir.AxisListType.X
Exp = mybir.ActivationFunctionType.Exp
Sqrt = mybir.ActivationFunctionType.Sqrt
Square = mybir.ActivationFunctionType.Square

B, S, HID = 1, 2048, 2048
H, KVH, HD = 16, 8, 128
NCORES = 8
HQ = H // NCORES            # 2 q heads per core
EPS = 1e-6
SCALE = float(np.float64(128.0) ** 0.5)   # reference divides by HD**-0.5
NT = S // 128               # 16 s-tiles
NQ = HQ * (S // 512)        # 8 qtiles (h, qt): 512 q's each

TRACE = False               # test.py flips this for timing runs
TRACE_DIR = None

_nc_cache = []


def build():
    nc = bass.Bass()
    hsT = nc.declare_dram_parameter("hsT", [NT, 128, 16, 128], F32R, isOutput=False)
    wqkv = nc.declare_dram_parameter("wqkv", [128, 16, 512], F32R, isOutput=False)
    wo = nc.declare_dram_parameter("wo", [128, HQ, 2048], BF16, isOutput=False)
    cosp = nc.declare_dram_parameter("cosp", [128, 16, 128], F32, isOutput=False)
    sinp = nc.declare_dram_parameter("sinp", [128, 16, 128], F32, isOutput=False)
    identp = nc.declare_dram_parameter("identp", [128, 128], F32, isOutput=False)
    constp = nc.declare_dram_parameter("constp", [128, 2], F32, isOutput=False)  # eps, 0
    ones1p = nc.declare_dram_parameter("ones1p", [1, 128], F32R, isOutput=False)
    onespp = nc.declare_dram_parameter("onespp", [128, 1], F32R, isOutput=False)
    out = nc.declare_dram_parameter("out", [S, HID], BF16, isOutput=True)

    es = contextlib.ExitStack()

    def sb(name, shape, dt):
        return es.enter_context(nc.sbuf_tensor(name, shape, dt))

    def psum(name, shape, dt):
        return es.enter_context(nc.psum_tensor(name, shape, dt))

    def sem(name):
        return es.enter_context(nc.semaphore(name))

    with es:
        # ---- SBUF ----
        hs_sb = [sb(f"hs{p}", [128, 16, 128], F32R) for p in range(4)]
        wqkv_sb = sb("wqkv_sb", [128, 16, 512], F32R)
        wo_sb = sb("wo_sb", [128, HQ, 2048], BF16)
        cos_sb = sb("cos_sb", [128, 16, 128], F32)
        sin_sb = sb("sin_sb", [128, 16, 128], F32)
        ident = sb("ident", [128, 128], F32)
        eps_t = sb("eps_t", [128, 2], F32)     # col0 eps, col1 zero
        ones1 = sb("ones1", [1, 128], F32R)
        onesp = sb("onesp", [128, 1], F32R)
        ssq = [sb(f"ssq{p}", [128, 3], F32) for p in range(2)]
        std = [sb(f"std{p}", [128, 3], F32) for p in range(2)]
        rstd = [sb(f"rstd{p}", [128, 3], F32) for p in range(2)]
        sq_scr = sb("sq_scr", [128, 3, 128], F32)
        qn = sb("qn", [128, 384], F32)
        tmp1 = sb("tmp1", [128, 384], F32)
        tmp2 = sb("tmp2", [128, 384], F32)
        qrope = [sb(f"qrope{p}", [128, 384], F32) for p in range(3)]
        qT = sb("qT", [128, HQ, S], F32R)
        kT = sb("kT", [128, S], F32R)
        v_sb = sb("v_sb", [128, 16, 128], BF16)
        attnT = sb("attnT", [128, HQ, S], BF16)
        gm4 = [sb(f"gm4_{p}", [128, 4], F32) for p in range(2)]
        negm_row = [sb(f"negm_row{p}", [1, 512], F32R) for p in range(2)]
        rinv_row = [sb(f"rinv_row{p}", [1, 512], F32R) for p in range(2)]
        rinvb_sb = [sb(f"rinvb{p}", [128, 512], F32) for p in range(2)]
        racc = [sb(f"racc{p}", [128, 512], F32R) for p in range(2)]
        r_a = sb("r_a", [128, 512], F32)
        r_b = sb("r_b", [128, 512], F32)
        r_c = sb("r_c", [128, 512], F32)
        r_d = sb("r_d", [128, 512], F32)
        probsT = [sb(f"probsT{p}", [128, 16, 512], BF16) for p in range(2)]
        out_sb = [sb(f"out_sb{p}", [128, 2048], BF16) for p in range(2)]

        # ---- PSUM (8 banks total) ----
        ps_qkv = [psum(f"ps_qkv{p}", [128, 512], F32) for p in range(2)]  # P + A(s2) + O
        sc = psum("sc", [128, 2048], F32)     # P(qkv 2,3) + A(s1, rsum/rinvb) + O
        ps_tr = psum("ps_tr", [128, 512], F32)   # P(tr) + A(av bank 0 / negmT even)
        ps_av2 = psum("ps_av2", [128, 512], F32)  # A(av bank 1 / negmT odd)
        av_banks = [ps_tr, ps_av2]

        # ---- semaphores ----
        s_hin = [sem("s_hin0"), sem("s_hin1"), sem("s_hin2"), sem("s_hin3")]
        s_const = sem("s_const"); s_cst = sem("s_cst")
        s_wqa = sem("s_wqa"); s_wqb = sem("s_wqb"); s_wo = sem("s_wo")
        s_qkv = sem("s_qkv");   s_sq = sem("s_sq");     s_std = sem("s_std")
        s_rstd = sem("s_rstd"); s_qn = sem("s_qn");     s_vcp = sem("s_vcp")
        s_ropem = sem("s_ropem"); s_roped = sem("s_roped")
        s_trmm = sem("s_trmm"); s_trcp = sem("s_trcp")
        s_s1 = sem("s_s1");     s_red = sem("s_red")
        s_negmT = sem("s_negmT"); s_nrow = sem("s_nrow")
        s_s2 = sem("s_s2");     s_exp = sem("s_exp")
        s_radd = sem("s_radd"); s_rsum = sem("s_rsum")
        s_rinv = sem("s_rinv"); s_rinvb = sem("s_rinvb"); s_rinvcp = sem("s_rinvcp")
        s_av = sem("s_av");     s_att = sem("s_att")
        s_omm = sem("s_omm");   s_ocp = sem("s_ocp");   s_ocp2 = sem("s_ocp2")
        s_outd = [sem("s_outd0"), sem("s_outd1")]

        block = es.enter_context(nc.Block())

        qkv_banks = [ps_qkv[0][:], ps_qkv[1][:], sc[:, 0:512], sc[:, 512:1024]]

        # qtile order: h-major -> Q = h*4 + qt
        def qtile(Q):
            return Q // 4, (Q % 4)   # (h, qt)

        # ---------------- SYNC: hs/wqkv stream + out stores ----------------
        # NOTE: DMA completions within a queue are unordered; every wait on a
        # DMA semaphore must cover ALL dmas that increment it.
        @block.sync
        def _(sy):
            sy.dma_start(out=hs_sb[0][:], in_=hsT[0]).then_inc(s_hin[0], 16)
            for c in range(8):
                sy.dma_start(out=wqkv_sb[:, c, :], in_=wqkv[:, c, :]).then_inc(s_wqa, 16)
            for c in range(8, 16):
                sy.dma_start(out=wqkv_sb[:, c, :], in_=wqkv[:, c, :]).then_inc(s_wqb, 16)
            for st in range(1, NT):
                if st >= 4:
                    sy.wait_ge(s_qkv, st - 3)          # hs buf consumed
                sy.dma_start(out=hs_sb[st % 4][:], in_=hsT[st]).then_inc(s_hin[st % 4], 16)
            # phase O: partial out stores
            for st in range(NT):
                sy.wait_ge(s_ocp, 2 * (st + 1))
                sy.wait_ge(s_ocp2, 2 * (st + 1))
                sy.dma_start(out=out[st * 128:(st + 1) * 128, :],
                             in_=out_sb[st % 2][:]).then_inc(s_outd[st % 2], 16)

        # ---------------- TENSOR ----------------
        @block.tensor
        def _(te):
            # ---- phase P ----
            def _tr_group(pst):
                te.wait_ge(s_roped, pst + 1)
                if pst >= 1:
                    te.wait_ge(s_trcp, pst)            # ps_tr bank: copies of pst-1 done
                for idx in range(3):
                    te.matmul(ps_tr[:, idx * 128:(idx + 1) * 128],
                              qrope[pst % 3][:, idx * 128:(idx + 1) * 128],
                              ident[:], is_transpose=True,
                              start=True, stop=True).then_maybe_inc(
                                  (s_trmm, 1) if idx == 2 else None)

            te.wait_ge(s_const, 64)                    # ident/ones/eps
            for st in range(NT):
                te.wait_ge(s_hin[st % 4], 16 * (st // 4 + 1))
                if st >= 4:
                    te.wait_ge(s_qn, st - 3)           # psum buf: qn scale read done
                    te.wait_ge(s_vcp, st - 3)          # psum buf: v copy done
                pq = qkv_banks[st % 4]
                for c in range(16):
                    if st == 0 and c == 0:
                        te.wait_ge(s_wqa, 128)
                    if st == 0 and c == 8:
                        te.wait_ge(s_wqb, 128)
                    te.matmul(pq[:], hs_sb[st % 4][:, c, :], wqkv_sb[:, c, :],
                              start=(c == 0), stop=(c == 15)).then_maybe_inc(
                                  (s_qkv, 1) if c == 15 else None)
                if st >= 2:
                    _tr_group(st - 2)
            _tr_group(NT - 2)
            _tr_group(NT - 1)

            # ---- phase A ----
            te.wait_ge(s_trcp, NT)                     # qT/kT complete
            te.wait_ge(s_qn, NT)                       # all P psum readers done
            te.wait_ge(s_vcp, NT)

            def _s1(Qn, il):
                # scores pass1 for i = (Qn, il): [128q, 2048k] into sc
                idx = 4 * Qn + il
                if idx >= 1:
                    te.wait_ge(s_red, idx)             # previous il's reduce done
                if Qn >= 2 and il == 0:
                    te.wait_ge(s_rinvcp, Qn - 1)       # rinvb copy out of sc[0:512]
                h, qt = qtile(Qn)
                q0 = qt * 512 + il * 128
                for b in range(4):
                    te.matmul(sc[:, b * 512:(b + 1) * 512],
                              qT[:, h, q0:q0 + 128],
                              kT[:, b * 512:(b + 1) * 512],
                              start=True, stop=True).then_maybe_inc(
                                  (s_s1, 1) if b == 3 else None)

            def _negmT(Qn):
                te.wait_ge(s_red, 4 * (Qn + 1))        # all 4 reduces of Qn done
                if Qn >= 2:
                    te.wait_ge(s_att, Qn - 1)          # av bank free (T_T of Qn-2 done)
                for il in range(4):
                    te.matmul(av_banks[Qn % 2][0:1, il * 128:(il + 1) * 128],
                              gm4[Qn % 2][:, il:il + 1],
                              ident[:], is_transpose=True,
                              start=True, stop=True).then_maybe_inc(
                                  (s_negmT, 1) if il == 3 else None)

            def _s2(Q, kb):
                h, qt = qtile(Q)
                if kb == 0:
                    te.wait_ge(s_nrow, Q + 1)          # negm_row(Q) ready
                if Q >= 1 or kb >= 2:
                    te.wait_ge(s_exp, 16 * Q + kb - 1)  # bank free: exp(Q,kb-2)
                bank = ps_qkv[kb % 2]
                te.matmul(bank[:], kT[:, kb * 128:(kb + 1) * 128],
                          qT[:, h, qt * 512:qt * 512 + 512],
                          start=True, stop=False)
                te.matmul(bank[:], ones1[:], negm_row[Q % 2][:],
                          start=False, stop=True).then_inc(s_s2, 1)

            def _av(Q, kb):
                h, qt = qtile(Q)
                te.wait_ge(s_exp, 16 * Q + kb + 1)
                if kb == 0 and Q >= 2:
                    te.wait_ge(s_att, Q - 1)           # av bank free
                te.matmul(av_banks[Q % 2][:], v_sb[:, kb, :],
                          probsT[Q % 2][:, kb, :],
                          start=(kb == 0), stop=(kb == 15),
                          skip_group_check=True).then_maybe_inc(
                              (s_av, 1) if kb == 15 else None)

            def _rsum(Qp):
                te.wait_ge(s_radd, 15 * (Qp + 1))      # racc(Qp) ready
                te.wait_ge(s_red, 4 * min(Qp + 2, NQ))  # sc free of pass1 readers
                if Qp >= 1:
                    te.wait_ge(s_rinvcp, Qp)           # sc[0:512] free of rinvb copy
                te.matmul(sc[0:1, 0:512], onesp[:], racc[Qp % 2][:],
                          start=True, stop=True).then_inc(s_rsum, 1)

            def _rinvb(Qp):
                te.wait_ge(s_rinv, Qp + 1)             # reciprocal row ready
                te.matmul(sc[:, 0:512], ones1[:], rinv_row[Qp % 2][:],
                          start=True, stop=True).then_inc(s_rinvb, 1)

            # prologue: s1(0) + negmT(0)
            for il in range(4):
                _s1(0, il)
            _negmT(0)

            for Q in range(NQ):
                # sc[0:512] lifetime: rsum W -> recip R (DVE) -> rinvb W ->
                # rinvcp R (DVE) -> s1 il0..3 W (each reduced before the next)
                for kb in range(4):
                    _s2(Q, kb)
                if Q >= 1:
                    _rsum(Q - 1)
                for kb in range(4, 8):
                    _s2(Q, kb)
                if Q >= 1:
                    _rinvb(Q - 1)
                for kb in range(8, 12):
                    _s2(Q, kb)
                if Q < NQ - 1:
                    _s1(Q + 1, 0)
                for kb in range(12, 16):
                    _s2(Q, kb)
                if Q < NQ - 1:
                    _s1(Q + 1, 1)
                for kb in range(8):
                    _av(Q, kb)
                if Q < NQ - 1:
                    _s1(Q + 1, 2)
                    _s1(Q + 1, 3)
                for kb in range(8, 16):
                    _av(Q, kb)
                if Q < NQ - 1:
                    _negmT(Q + 1)
            _rsum(NQ - 1)
            _rinvb(NQ - 1)

            # ---- phase O ----
            te.wait_ge(s_att, NQ)
            te.wait_ge(s_rinvcp, NQ)
            te.wait_ge(s_wo, 16)                       # wo loaded
            o_banks = [ps_qkv[0][:], ps_qkv[1][:], sc[:, 0:512], sc[:, 512:1024]]
            for st in range(NT):
                for eb in range(4):
                    idx = st * 4 + eb
                    if idx >= 4:
                        pidx = idx - 4
                        if pidx % 2 == 0:
                            te.wait_ge(s_ocp, pidx // 2 + 1)
                        else:
                            te.wait_ge(s_ocp2, pidx // 2 + 1)
                    po = o_banks[idx % 4]
                    for h in range(HQ):
                        te.matmul(po[:], attnT[:, h, st * 128:(st + 1) * 128],
                                  wo_sb[:, h, eb * 512:(eb + 1) * 512],
                                  start=(h == 0), stop=(h == HQ - 1)).then_maybe_inc(
                                      (s_omm, 1) if h == HQ - 1 else None)

        # ---------------- SCALAR (ACT) ----------------
        @block.scalar
        def _(se):
            def _tr_copy(st):
                se.wait_ge(s_trmm, st + 1)
                se.copy(out=qT[:, :, st * 128:(st + 1) * 128],
                        in_=ps_tr[:, 0:256].rearrange("p (h d) -> p h d", h=2))
                se.copy(out=kT[:, st * 128:(st + 1) * 128],
                        in_=ps_tr[:, 256:384]).then_inc(s_trcp, 1)

            se.wait_ge(s_const, 64)                    # consts loaded
            # ---- phase P ----
            for st in range(NT):
                se.wait_ge(s_qkv, st + 1)
                if st >= 2:
                    se.wait_ge(s_rstd, st - 1)         # std buf: recip read done
                pq = qkv_banks[st % 4]
                for hh in range(3):
                    se.activation(out=sq_scr[:, hh, :], in_=pq[:, hh * 128:(hh + 1) * 128],
                                  func=Square,
                                  accum_out=ssq[st % 2][:, hh:hh + 1]).then_maybe_inc(
                                      (s_sq, 1) if hh == 2 else None)
                se.wait_ge(s_sq, st + 1)               # own squares done
                se.activation(out=std[st % 2][:], in_=ssq[st % 2][:], func=Sqrt,
                              scale=1.0 / 128.0, bias=eps_t[:, 0:1]).then_inc(s_std, 1)
                # v copy (frees psum buf together with s_qn)
                se.copy(out=v_sb[:, st, :], in_=pq[:, 384:512]).then_inc(s_vcp, 1)
                if st >= 2:
                    _tr_copy(st - 2)

            _tr_copy(NT - 2)
            _tr_copy(NT - 1)

            # ---- phase A ----
            for Q in range(NQ):
                # negm row copy from PE-transposed gm4 (psum av bank Q%2, partition 0)
                se.wait_ge(s_negmT, Q + 1)
                se.copy(out=negm_row[Q % 2][:],
                        in_=av_banks[Q % 2][0:1, 0:512]).then_inc(s_nrow, 1)
                for kb in range(16):
                    se.wait_ge(s_s2, 16 * Q + kb + 1)
                    if kb == 0 and Q >= 2:
                        se.wait_ge(s_radd, 15 * (Q - 1))  # probsT buf: gpsimd of Q-2 done
                        se.wait_ge(s_av, Q - 1)        # probsT buf: AV of Q-2 done
                    se.activation(out=probsT[Q % 2][:, kb, :],
                                  in_=ps_qkv[kb % 2][:], func=Exp,
                                  bias=eps_t[:, 1:2]).then_inc(s_exp, 1)

            # ---- phase O ----
            for st in range(NT):
                for eb in range(4):
                    idx = st * 4 + eb
                    if eb % 2 != 0:
                        continue
                    se.wait_ge(s_omm, idx + 1)
                    if eb == 0 and st >= 2:
                        se.wait_ge(s_outd[st % 2], 16 * (st // 2))  # out_sb buf free
                    se.copy(out=out_sb[st % 2][:, eb * 512:(eb + 1) * 512],
                            in_=[ps_qkv[0][:], ps_qkv[1][:], sc[:, 0:512],
                                 sc[:, 512:1024]][idx % 4]).then_inc(s_ocp, 1)

        # ---------------- VECTOR (DVE) ----------------
        @block.vector
        def _(ve):
            # ---- phase P ----
            for st in range(NT):
                ve.wait_ge(s_std, st + 1)
                ve.reciprocal(out=rstd[st % 2][:], in_=std[st % 2][:]).then_inc(s_rstd, 1)
                ve.wait_ge(s_rstd, st + 1)             # self RAW
                pq = qkv_banks[st % 4]
                for hh in range(3):
                    if hh < 2:   # q heads: also fold in the softmax scale sqrt(HD)
                        ve.tensor_scalar(out=qn[:, hh * 128:(hh + 1) * 128],
                                         in0=pq[:, hh * 128:(hh + 1) * 128],
                                         scalar1=rstd[st % 2][:, hh:hh + 1],
                                         scalar2=SCALE,
                                         op0=mybir.AluOpType.mult,
                                         op1=mybir.AluOpType.mult)
                    else:
                        ve.tensor_scalar_mul(out=qn[:, hh * 128:(hh + 1) * 128],
                                             in0=pq[:, hh * 128:(hh + 1) * 128],
                                             scalar1=rstd[st % 2][:, hh:hh + 1]).then_inc(s_qn, 1)
                ve.wait_ge(s_qn, st + 1)               # self RAW on qn
                if st >= 3:
                    ve.wait_ge(s_trmm, st - 2)         # qrope buf consumed by PE
                if st == 0:
                    ve.wait_ge(s_cst, 32)              # cos/sin loaded
                ct = cos_sb[:, st, :]
                stt = sin_sb[:, st, :]
                for hh in range(3):
                    c0 = hh * 128
                    ve.tensor_mul(out=tmp1[:, c0:c0 + 128], in0=qn[:, c0:c0 + 128], in1=ct)
                for hh in range(3):
                    c0 = hh * 128
                    ve.tensor_mul(out=tmp2[:, c0:c0 + 64],
                                  in0=qn[:, c0 + 64:c0 + 128], in1=stt[:, 0:64])
                    ve.tensor_mul(out=tmp2[:, c0 + 64:c0 + 128],
                                  in0=qn[:, c0:c0 + 64],
                                  in1=stt[:, 64:128]).then_maybe_inc(
                                      (s_ropem, 1) if hh == 2 else None)
                ve.wait_ge(s_ropem, st + 1)            # self RAW on tmp1/tmp2
                qr = qrope[st % 3]
                for hh in range(3):
                    c0 = hh * 128
                    ve.tensor_sub(out=qr[:, c0:c0 + 64],
                                  in0=tmp1[:, c0:c0 + 64], in1=tmp2[:, c0:c0 + 64])
                    ve.tensor_add(out=qr[:, c0 + 64:c0 + 128],
                                  in0=tmp1[:, c0 + 64:c0 + 128],
                                  in1=tmp2[:, c0 + 64:c0 + 128]).then_maybe_inc(
                                      (s_roped, 1) if hh == 2 else None)

            # ---- phase A ----
            def _reduce(Qn, il):
                idx = 4 * Qn + il
                ve.wait_ge(s_s1, idx + 1)
                if Qn >= 2 and il == 0:
                    ve.wait_ge(s_negmT, Qn - 1)        # gm4 buf free
                ve.reduce_max(out=gm4[Qn % 2][:, il:il + 1], in_=sc[:, 0:2048],
                              axis=AX, negate=True).then_inc(s_red, 1)

            # prologue reduces for Q=0
            for il in range(4):
                _reduce(0, il)

            for Q in range(NQ):
                if Q >= 1:
                    Qp = Q - 1
                    ve.wait_ge(s_rsum, Qp + 1)
                    with nc.allow_low_precision(reason="rinv row in f32r"):
                        ve.reciprocal(out=rinv_row[Qp % 2][:], in_=sc[0:1, 0:512]).then_inc(s_rinv, 1)
                    ve.wait_ge(s_rinvb, Qp + 1)
                    ve.tensor_copy(out=rinvb_sb[Qp % 2][:], in_=sc[:, 0:512]).then_inc(s_rinvcp, 1)
                    hp, qtp = qtile(Qp)
                    ve.wait_ge(s_rinvcp, Qp + 1)       # self RAW on rinvb_sb
                    ve.wait_ge(s_av, Qp + 1)
                    ve.tensor_mul(out=attnT[:, hp, qtp * 512:qtp * 512 + 512],
                                  in0=av_banks[Qp % 2][:],
                                  in1=rinvb_sb[Qp % 2][:]).then_inc(s_att, 1)
                if Q < NQ - 1:
                    _reduce(Q + 1, 0)
                    _reduce(Q + 1, 1)
                    _reduce(Q + 1, 2)
                    _reduce(Q + 1, 3)
            # epilogue: Q = NQ-1 tail
            Qp = NQ - 1
            ve.wait_ge(s_rsum, Qp + 1)
            with nc.allow_low_precision(reason="rinv row in f32r"):
                ve.reciprocal(out=rinv_row[Qp % 2][:], in_=sc[0:1, 0:512]).then_inc(s_rinv, 1)
            ve.wait_ge(s_rinvb, Qp + 1)
            ve.tensor_copy(out=rinvb_sb[Qp % 2][:], in_=sc[:, 0:512]).then_inc(s_rinvcp, 1)
            hp, qtp = qtile(Qp)
            ve.wait_ge(s_rinvcp, Qp + 1)       # self RAW on rinvb_sb
            ve.wait_ge(s_av, Qp + 1)
            ve.tensor_mul(out=attnT[:, hp, qtp * 512:qtp * 512 + 512],
                          in0=av_banks[Qp % 2][:],
                          in1=rinvb_sb[Qp % 2][:]).then_inc(s_att, 1)

            # ---- phase O: odd-eb copies on DVE ----
            for st in range(NT):
                for eb in range(4):
                    idx = st * 4 + eb
                    if eb % 2 != 1:
                        continue
                    ve.wait_ge(s_omm, idx + 1)
                    if eb == 1 and st >= 2:
                        ve.wait_ge(s_outd[st % 2], 16 * (st // 2))  # out_sb buf free
                    ve.tensor_copy(out=out_sb[st % 2][:, eb * 512:(eb + 1) * 512],
                                   in_=[ps_qkv[0][:], ps_qkv[1][:], sc[:, 0:512],
                                        sc[:, 512:1024]][idx % 4]).then_inc(s_ocp2, 1)

        # ---------------- GPSIMD: const DMAs + probsT row-sum trees ----------------
        @block.gpsimd
        def _(gp):
            # second DMA queue: small consts + cos/sin + wo (parallel to sync's
            # hs/wqkv stream)
            gp.dma_start(out=eps_t[:], in_=constp[:]).then_inc(s_const, 16)
            gp.dma_start(out=ident[:], in_=identp[:]).then_inc(s_const, 16)
            gp.dma_start(out=ones1[:], in_=ones1p[:]).then_inc(s_const, 16)
            gp.dma_start(out=onesp[:], in_=onespp[:]).then_inc(s_const, 16)
            gp.dma_start(out=cos_sb[:], in_=cosp[:]).then_inc(s_cst, 16)
            gp.dma_start(out=sin_sb[:], in_=sinp[:]).then_inc(s_cst, 16)
            gp.dma_start(out=wo_sb[:], in_=wo[:]).then_inc(s_wo, 16)
            for Q in range(NQ):
                pT = probsT[Q % 2]
                b = 15 * Q
                # pairwise tree; every add incs s_radd; self-RAW via waits
                if Q >= 1:
                    gp.wait_ge(s_radd, b)              # WAR: r_a/r_c read by Q-1 tail
                gp.wait_ge(s_exp, 16 * Q + 2)
                gp.tensor_add(out=r_a[:], in0=pT[:, 0, :], in1=pT[:, 1, :]).then_inc(s_radd, 1)
                gp.wait_ge(s_exp, 16 * Q + 4)
                gp.tensor_add(out=r_b[:], in0=pT[:, 2, :], in1=pT[:, 3, :]).then_inc(s_radd, 1)
                gp.wait_ge(s_exp, 16 * Q + 6)
                gp.tensor_add(out=r_c[:], in0=pT[:, 4, :], in1=pT[:, 5, :]).then_inc(s_radd, 1)
                gp.wait_ge(s_exp, 16 * Q + 8)
                gp.tensor_add(out=r_d[:], in0=pT[:, 6, :], in1=pT[:, 7, :]).then_inc(s_radd, 1)
                gp.wait_ge(s_radd, b + 2)
                gp.tensor_add(out=r_a[:], in0=r_a[:], in1=r_b[:]).then_inc(s_radd, 1)
                gp.wait_ge(s_radd, b + 4)
                gp.tensor_add(out=r_c[:], in0=r_c[:], in1=r_d[:]).then_inc(s_radd, 1)
                gp.wait_ge(s_radd, b + 5)              # WAR: r_b read by op5
                gp.wait_ge(s_exp, 16 * Q + 10)
                gp.tensor_add(out=r_b[:], in0=pT[:, 8, :], in1=pT[:, 9, :]).then_inc(s_radd, 1)
                gp.wait_ge(s_radd, b + 6)              # WAR: r_d read by op6
                gp.wait_ge(s_exp, 16 * Q + 12)
                gp.tensor_add(out=r_d[:], in0=pT[:, 10, :], in1=pT[:, 11, :]).then_inc(s_radd, 1)
                gp.wait_ge(s_radd, b + 8)
                gp.tensor_add(out=r_b[:], in0=r_b[:], in1=r_d[:]).then_inc(s_radd, 1)
                gp.wait_ge(s_radd, b + 6)
                gp.tensor_add(out=r_a[:], in0=r_a[:], in1=r_c[:]).then_inc(s_radd, 1)
                gp.wait_ge(s_radd, b + 10)             # WAR: r_c read by op10
                gp.wait_ge(s_exp, 16 * Q + 14)
                gp.tensor_add(out=r_c[:], in0=pT[:, 12, :], in1=pT[:, 13, :]).then_inc(s_radd, 1)
                gp.wait_ge(s_exp, 16 * Q + 16)
                gp.tensor_add(out=r_d[:], in0=pT[:, 14, :], in1=pT[:, 15, :]).then_inc(s_radd, 1)
                gp.wait_ge(s_radd, b + 12)
                gp.tensor_add(out=r_c[:], in0=r_c[:], in1=r_d[:]).then_inc(s_radd, 1)
                gp.wait_ge(s_radd, b + 10)
                gp.tensor_add(out=r_a[:], in0=r_a[:], in1=r_b[:]).then_inc(s_radd, 1)
                gp.wait_ge(s_radd, b + 14)
                if Q >= 2:
                    gp.wait_ge(s_rsum, Q - 1)          # racc buf read by rsum(Q-2)
                gp.tensor_add(out=racc[Q % 2][:], in0=r_a[:], in1=r_c[:]).then_inc(s_radd, 1)

    return nc


def _host_prep(hidden_states, cos, sin, wq, wk, wv, wo):
    hs = np.ascontiguousarray(hidden_states.reshape(S, HID), dtype=np.float32)
    # hsT tiles: t[st, p, c, s] = hs[st*128+s, c*128+p]
    hsT = np.ascontiguousarray(
        hs.reshape(NT, 128, 16, 128).transpose(0, 3, 2, 1))
    cos_t = np.ascontiguousarray(
        cos.reshape(NT, 128, HD).transpose(1, 0, 2), dtype=np.float32)
    sin_t = np.ascontiguousarray(
        sin.reshape(NT, 128, HD).transpose(1, 0, 2), dtype=np.float32)
    ident = np.eye(128, dtype=np.float32)
    const_c = np.zeros((128, 2), dtype=np.float32)
    const_c[:, 0] = EPS
    ones1 = np.ones((1, 128), dtype=np.float32)
    onesp = np.ones((128, 1), dtype=np.float32)

    in_maps = []
    for g in range(NCORES):
        wq_g = wq[:, g * HQ * HD:(g + 1) * HQ * HD]          # [2048, 256]
        wk_g = wk[:, g * HD:(g + 1) * HD]                    # [2048, 128]
        wv_g = wv[:, g * HD:(g + 1) * HD]                    # [2048, 128]
        wqkv_g = np.concatenate([wq_g, wk_g, wv_g], axis=1)  # [2048, 512]
        wqkv_t = np.ascontiguousarray(
            wqkv_g.reshape(16, 128, 512).transpose(1, 0, 2), dtype=np.float32)
        wo_g = wo[g * HQ * HD:(g + 1) * HQ * HD, :]          # [256, 2048]
        wo_t = np.ascontiguousarray(
            wo_g.reshape(HQ, 128, HID).transpose(1, 0, 2)).astype(ml_dtypes.bfloat16)
        in_maps.append({
            "hsT": hsT, "wqkv": wqkv_t, "wo": wo_t,
            "cosp": cos_t, "sinp": sin_t, "identp": ident,
            "constp": const_c, "ones1p": ones1, "onespp": onesp,
        })
    return in_maps


def kernel(hidden_states, cos, sin, wq, wk, wv, wo):
    hidden_states = np.asarray(hidden_states, dtype=np.float32)
    cos = np.asarray(cos, dtype=np.float32).reshape(S, HD)
    sin = np.asarray(sin, dtype=np.float32).reshape(S, HD)
    wq = np.asarray(wq, dtype=np.float32)
    wk = np.asarray(wk, dtype=np.float32)
    wv = np.asarray(wv, dtype=np.float32)
    wo = np.asarray(wo, dtype=np.float32)

    in_maps = _host_prep(hidden_states, cos, sin, wq, wk, wv, wo)
    if not _nc_cache:
        _nc_cache.append(build())
    nc = _nc_cache[0]
    kw = {}
    if TRACE:
        import tempfile
        kw = dict(trace=True, tmpdir=tempfile.mkdtemp(prefix="attn_trace_"))
    res = bass_utils.run_bass_kernel_spmd(nc, in_maps, list(range(NCORES)), **kw)
    if TRACE:
        print("HW exec time: %d ns" % res.exec_time_ns)
    acc = np.zeros((S, HID), dtype=np.float32)
    for g in range(NCORES):
        acc += res.results[g]["out"].astype(np.float32)
    return acc.reshape(B, S, HID)
